# revision 1
# baseline (speedup 1.0000x reference)
"""Trainium2 Bass kernel for nn_FISLayerParameterSharingV1 (FIS layer, 2 tree families).

Strategy (8 NeuronCores, data-parallel over batch: 2 images/core):
  - All on-chip tensors keep W on partitions (2 chunks of 128) and H on the
    free axis, in 257-pitch "segment" layout (col 0 of each tree segment is a
    zero pad so one DVE scan instruction covers all 8 trees of a field with
    a multiplicative carry-kill mask at segment starts).
  - Channel contraction (einsum tc,bchw->bthw) on the PE in float32r:
    lhsT = x tile [(c32,dh8), w128], rhs = block-diag alpha [(c32,dh8),(t48,dh8)].
  - Directional prefix sums are separable: the H direction runs as one
    segmented exclusive scan (tensor_tensor_scan, fwd or reverse via
    negative-stride APs); the W direction is a strict-triangular f32r matmul
    (blocks [k<m], [k>m], ones).
  - Pointwise tree products on the vector engine (PSUM operand fused).
  - Final [w,h] -> [h,w] via PE transpose, ACT copy to SBUF, DMA out.

Measured single-image accuracy vs fp64 reference: ~6e-4 max relative.
"""
import sys
sys.path.insert(0, "/opt/trn_rl_repo")
import numpy as np
import concourse.bass as bass
import concourse.mybir as mybir
from concourse import tile
from concourse.vector_clock import ScopedClock
from concourse.bass_utils import run_bass_kernel_spmd

F32 = mybir.dt.float32
F32R = mybir.dt.float32r
AL = mybir.AluOpType

B, C, H, W = 16, 32, 256, 256
NCORES = 8
BPC = B // NCORES    # 2 images per core
T2 = 8
NF = 6
NT = NF * T2         # 48 stacked trees (field-major)
SEG = 257
FSPAN = T2 * SEG     # 2056
DH = 4               # h-pack per weighting matmul
NTP = 64             # padded tree count so weighting N=256 (f32r full rate)
NG = H // DH         # 32 h-groups


# ---------------------------------------------------------------------------
# Workarounds: this walrus build allows only ONE sync-wait per instruction.
# ---------------------------------------------------------------------------
def _patched_drain_and_barrier(self, tick_clock, wait_clock):
    nop_inst = self.nc.sync.nop(nofuse=True, hint="drain_waits")
    wait_clock.add_sem_waits(nop_inst.ins, ScopedClock({None: tick_clock.global_clock}))
    si = nop_inst.ins.sync_info
    waits = list(si.on_wait) if si and si.on_wait else []
    if len(waits) > 1:
        si.on_wait[:] = waits[:1]
        for w in waits[1:]:
            extra = self.nc.sync.nop(nofuse=True, hint="drain_waits")
            extra.ins.sync_info = mybir.SyncInfo(on_wait=[w], on_update=[])
    self.nc.sync.drain()
    self.nc.all_engine_barrier()
    assert self.sems is not None
    popped = self.nc._tile_sem_poison_stack.pop()
    assert popped is self._sem_poison
    self.nc.clear_and_free_semaphores(list(self.sems.allocated().values()))
    self.nc.all_engine_barrier()


tile.TileContext._drain_and_barrier = _patched_drain_and_barrier


def _split_multi_waits(nc):
    n_split = 0
    for bb in nc.main_func.blocks:
        insts = bb.instructions
        i = 0
        while i < len(insts):
            inst = insts[i]
            si = inst.sync_info
            if si is not None and si.on_wait and len(si.on_wait) > 1:
                waits = list(si.on_wait)
                si.on_wait[:] = waits[-1:]
                for j, w in enumerate(waits[:-1]):
                    ev = mybir.InstEventSemaphore(
                        name=f"{inst.name}_xw{j}", ins=[], outs=[])
                    ev.engine = inst.engine
                    ev.sync_info = mybir.SyncInfo(on_wait=[w], on_update=[])
                    insts.insert(i, ev)
                    i += 1
                    n_split += 1
            i += 1
    return n_split


# ---------------------------------------------------------------------------
# Bass program (SPMD, same for every core)
# ---------------------------------------------------------------------------
def build_program():
    nc = bass.Bass()
    x_in = nc.dram_tensor("x", [BPC, C, H, W], F32R, kind="ExternalInput")
    bd_in = nc.dram_tensor("bd", [128, NTP * DH], F32R, kind="ExternalInput")
    tri_in = nc.dram_tensor("tri", [3, 128, 128], F32R, kind="ExternalInput")
    msk_in = nc.dram_tensor("msk", [128, FSPAN + 1], F32, kind="ExternalInput")
    idn_in = nc.dram_tensor("idn", [128, 128], F32, kind="ExternalInput")
    y_out = nc.dram_tensor("y", [BPC, 2 * T2, H, W], F32, kind="ExternalOutput")

    TU = T2 // 2          # trees per family per unit (4)
    NFT = NF * TU         # segments per unit buffer (24)
    USPAN = NFT * SEG     # 6168
    FU = TU * SEG         # per-field span in a unit (1028)

    with tile.TileContext(nc) as tc:
        with tc.tile_pool(name="consts", bufs=1) as cpool, \
             tc.tile_pool(name="xp", bufs=6) as xp, \
             tc.tile_pool(name="fieldp", bufs=1) as fieldp, \
             tc.tile_pool(name="scoutp", bufs=8) as scoutp, \
             tc.tile_pool(name="m1p", bufs=2) as m1p, \
             tc.tile_pool(name="finp", bufs=3) as finp, \
             tc.tile_pool(name="outp", bufs=3) as outp, \
             tc.tile_pool(name="pw", bufs=4, space="PSUM") as pwp, \
             tc.tile_pool(name="pd", bufs=2, space="PSUM") as pdp, \
             tc.tile_pool(name="pt", bufs=2, space="PSUM") as ptp:

            bd = cpool.tile([128, NTP * DH], F32R, tag="bd", name="bd")
            nc.sync.dma_start(bd[:], bd_in[:])
            TT = cpool.tile([128, 128], F32R, tag="TT", name="TT")
            TP = cpool.tile([128, 128], F32R, tag="TP", name="TP")
            JJ = cpool.tile([128, 128], F32R, tag="JJ", name="JJ")
            nc.sync.dma_start(TT[:], tri_in[0])
            nc.sync.dma_start(TP[:], tri_in[1])
            nc.sync.dma_start(JJ[:], tri_in[2])
            MSK = cpool.tile([128, FSPAN + 1], F32, tag="MSK", name="MSK")
            nc.sync.dma_start(MSK[:], msk_in[:])
            IDN = cpool.tile([128, 128], F32, tag="IDN", name="IDN")
            nc.sync.dma_start(IDN[:], idn_in[:])

            def seg_scan(out_t, in_ap, rev):
                if not rev:
                    nc.vector.tensor_tensor_scan(
                        out_t[:, 1:FU], MSK[:, 1:FU], in_ap[:, 0:FU - 1],
                        0.0, op0=AL.mult, op1=AL.add)
                else:
                    nc.vector.memset(out_t[:, FU - 1:FU].bitcast(F32), 0.0)
                    nc.vector.tensor_tensor_scan(
                        out_t[:, FU - 2::-1], MSK[:, FU:1:-1],
                        in_ap[:, FU - 1:0:-1], 0.0, op0=AL.mult, op1=AL.add)

            for b in range(BPC):
                UF = [[fieldp.tile([128, USPAN], F32, tag=f"uf{u}{wc}",
                                   name="UF") for wc in range(2)]
                      for u in range(2)]
                V2 = [[fieldp.tile([128, FU], F32, tag=f"v2_{u}{wc}", name="V2")
                       for wc in range(2)] for u in range(2)]

                # ---- weighting: x[b] as [(c,hq) 128, (hi 64, w 256)] ----
                xv = x_in[b].rearrange("c (q i) w -> (c q) (i w)", q=4)
                for k in range(16):
                    xt = xp.tile([128, 4 * W], F32R, tag="xt", name="xt")
                    nc.gpsimd.dma_start(xt[:], xv[:, k * 4 * W:(k + 1) * 4 * W])
                    for jq in range(2):
                        for wc in range(2):
                            ps = pwp.tile([128, 2 * NTP * DH], F32, tag="pw",
                                          name="ps")
                            for j in range(2):
                                hi = jq * 2 + j
                                nc.tensor.matmul(
                                    ps[:, j * NTP * DH:(j + 1) * NTP * DH],
                                    xt[:, hi * W + wc * 128:hi * W + (wc + 1) * 128],
                                    bd[:], start=True, stop=True)
                            hi0 = k * 4 + jq * 2
                            for u in range(2):
                                # dest (j, ft, hq): col = ft*SEG + 1 + hq*64 + hi0 + j
                                full = UF[u][wc][:]
                                dst = bass.AP(
                                    full.tensor, full.offset + 1 + hi0,
                                    [[USPAN, 128], [1, 2], [SEG, NFT], [64, DH]])
                                src = ps[:].rearrange(
                                    "p (j n) -> p j n", j=2)[:, :, u * 96:(u + 1) * 96]
                                if jq % 2 == 0:
                                    nc.scalar.copy(dst, src)
                                else:
                                    nc.vector.tensor_copy(dst, src)

                # ---- z-col zeroing ----
                for u in range(2):
                    for wc in range(2):
                        nc.gpsimd.memset(
                            UF[u][wc][:].rearrange(
                                "p (t s) -> p t s", s=SEG)[:, :, 0:1], 0.0)
                        nc.gpsimd.memset(
                            V2[u][wc][:].rearrange(
                                "p (t s) -> p t s", s=SEG)[:, :, 0:1], 0.0)

                yv = y_out[b].rearrange("t (a p) w -> t p a w", a=2)

                for u in range(2):
                    def field_span(wc, f, u=u):
                        return UF[u][wc][:, f * FU:(f + 1) * FU]

                    def tri_mms(scout, w_rev, tp):
                        rhs = [scout[kc][:].rearrange("p (t s) -> p t s", s=SEG)
                               [:, 2 * tp:2 * tp + 2, 1:SEG] for kc in range(2)]
                        pd0 = pdp.tile([128, 512], F32, tag="pd", name="pd0")
                        pd1 = pdp.tile([128, 512], F32, tag="pd", name="pd1")
                        if not w_rev:
                            nc.tensor.matmul(pd0[:], TT[:], rhs[0], start=True, stop=True)
                            nc.tensor.matmul(pd1[:], JJ[:], rhs[0], start=True, stop=False)
                            nc.tensor.matmul(pd1[:], TT[:], rhs[1], start=False, stop=True)
                        else:
                            nc.tensor.matmul(pd0[:], TP[:], rhs[0], start=True, stop=False)
                            nc.tensor.matmul(pd0[:], JJ[:], rhs[1], start=False, stop=True)
                            nc.tensor.matmul(pd1[:], TP[:], rhs[1], start=True, stop=True)
                        return [pd0, pd1]

                    def seg_view(buf_ap, tp):
                        return buf_ap.rearrange(
                            "p (t s) -> p t s", s=SEG)[:, 2 * tp:2 * tp + 2, 1:SEG]

                    def emit_out(kind, tp, fin_m, u=u):
                        for tl in range(2):
                            tloc = 4 * u + 2 * tp + tl
                            pt = ptp.tile([128, 512], F32, tag="pt", name="pt")
                            for hc in range(2):
                                for m in range(2):
                                    nc.tensor.transpose(
                                        pt[:, 256 * hc + 128 * m:
                                           256 * hc + 128 * (m + 1)],
                                        fin_m[m][:, 256 * tl + 128 * hc:
                                                 256 * tl + 128 * (hc + 1)],
                                        IDN[:])
                            ot = outp.tile([128, 512], F32, tag="ot", name="ot")
                            nc.scalar.copy(ot[:], pt[:])
                            nc.sync.dma_start(
                                yv[kind * T2 + tloc],
                                ot[:].rearrange("p (a w) -> p a w", a=2))

                    # ---- H scans of dirsum inputs ----
                    SC_C = [scoutp.tile([128, FU], F32R, tag="sc", name="SC")
                            for _ in range(2)]
                    for wc in range(2):
                        seg_scan(SC_C[wc], field_span(wc, 2), rev=True)
                    SC_Cp = [scoutp.tile([128, FU], F32R, tag="sc", name="SCp")
                             for _ in range(2)]
                    for wc in range(2):
                        seg_scan(SC_Cp[wc], field_span(wc, 5), rev=True)
                    SC_B2p = [scoutp.tile([128, FU], F32R, tag="sc", name="SCb")
                              for _ in range(2)]
                    for wc in range(2):
                        seg_scan(SC_B2p[wc], field_span(wc, 4), rev=True)

                    # ---- SW(C) + v2 (gpsimd muls, ACT-staged PD) ----
                    for tp in range(2):
                        pds = tri_mms(SC_C, w_rev=False, tp=tp)
                        for m in range(2):
                            stg = m1p.tile([128, 512], F32, tag="stg", name="stg")
                            nc.scalar.copy(stg[:], pds[m][:])
                            nc.gpsimd.tensor_tensor(
                                seg_view(V2[u][m][:], tp),
                                stg[:].rearrange("p (t h) -> p t h", t=2),
                                seg_view(field_span(m, 1), tp), op=AL.mult)

                    # ---- cherry ----
                    for tp in range(2):
                        finm = [finp.tile([128, 512], F32, tag="fin", name="FINC")
                                for _ in range(2)]
                        pds = tri_mms(SC_Cp, w_rev=False, tp=tp)
                        m1t = []
                        for m in range(2):
                            stg = m1p.tile([128, 512], F32, tag="stg", name="stg")
                            nc.scalar.copy(stg[:], pds[m][:])
                            t = m1p.tile([128, 512], F32, tag="m1", name="m1")
                            nc.gpsimd.tensor_tensor(
                                t[:].rearrange("p (t h) -> p t h", t=2),
                                stg[:].rearrange("p (t h) -> p t h", t=2),
                                seg_view(field_span(m, 3), tp), op=AL.mult)
                            m1t.append(t)
                        pds2 = tri_mms(SC_B2p, w_rev=True, tp=tp)
                        for m in range(2):
                            if m == 0:
                                stg2 = m1p.tile([128, 512], F32, tag="stg",
                                                name="stg2")
                                nc.scalar.copy(stg2[:], pds2[m][:])
                                nc.gpsimd.tensor_tensor(
                                    finm[m][:], stg2[:], m1t[m][:], op=AL.mult)
                            else:
                                nc.vector.tensor_tensor(
                                    finm[m][:], pds2[m][:], m1t[m][:], op=AL.mult)
                        emit_out(1, tp, finm)

                    # ---- linear ----
                    SC_V2 = [scoutp.tile([128, FU], F32R, tag="sc", name="SCv")
                             for _ in range(2)]
                    for wc in range(2):
                        seg_scan(SC_V2[wc], V2[u][wc][:], rev=False)
                    for tp in range(2):
                        finm = [finp.tile([128, 512], F32, tag="fin", name="FINL")
                                for _ in range(2)]
                        pds = tri_mms(SC_V2, w_rev=True, tp=tp)
                        for m in range(2):
                            if m == 0:
                                stg3 = m1p.tile([128, 512], F32, tag="stg",
                                                name="stg3")
                                nc.scalar.copy(stg3[:], pds[m][:])
                                nc.gpsimd.tensor_tensor(
                                    finm[m][:].rearrange("p (t h) -> p t h", t=2),
                                    stg3[:].rearrange("p (t h) -> p t h", t=2),
                                    seg_view(field_span(m, 0), tp), op=AL.mult)
                            else:
                                nc.vector.tensor_tensor(
                                    finm[m][:].rearrange("p (t h) -> p t h", t=2),
                                    pds[m][:].rearrange("p (t h) -> p t h", t=2),
                                    seg_view(field_span(m, 0), tp), op=AL.mult)
                        emit_out(0, tp, finm)

    _split_multi_waits(nc)
    return nc


def build_consts(alpha_1, alpha_2, alpha_3, alpha_1p, alpha_2p, alpha_3p):
    alpha = np.concatenate(
        [alpha_1, alpha_2, alpha_3, alpha_1p, alpha_2p, alpha_3p], 0
    ).astype(np.float32)                       # [48, 32] field-major
    # rows p = (c, hq): p = c*4 + hq; cols n = (u, f, t_l, hq'):
    # n = u*96 + f*16 + t_l*4 + hq', tree t_global = f*T2 + 4u + t_l
    bdm = np.zeros((128, NTP * DH), dtype=np.float32)
    for c in range(C):
        for hq in range(DH):
            for u in range(2):
                for f in range(NF):
                    for tl in range(4):
                        bdm[c * DH + hq, u * 96 + f * 16 + tl * 4 + hq] = \
                            alpha[f * T2 + 4 * u + tl, c]
    k = np.arange(128)
    tri = np.stack([
        (k[:, None] < k[None, :]).astype(np.float32),
        (k[:, None] > k[None, :]).astype(np.float32),
        np.ones((128, 128), np.float32)])
    msk = np.ones((128, FSPAN + 1), dtype=np.float32)
    zc = [j for j in range(FSPAN + 1) if j % SEG == 1]
    msk[:, zc] = 0.0
    idn = np.eye(128, dtype=np.float32)
    return bdm, tri, msk, idn


_CACHED = {}


def kernel(x, alpha_1, alpha_2, alpha_3, alpha_1p, alpha_2p, alpha_3p,
           _trace=False):
    x = np.ascontiguousarray(np.asarray(x, dtype=np.float32))
    bdm, tri, msk, idn = build_consts(
        np.asarray(alpha_1, np.float32), np.asarray(alpha_2, np.float32),
        np.asarray(alpha_3, np.float32), np.asarray(alpha_1p, np.float32),
        np.asarray(alpha_2p, np.float32), np.asarray(alpha_3p, np.float32))
    if "nc" not in _CACHED:
        _CACHED["nc"] = build_program()
    nc = _CACHED["nc"]
    in_maps = [
        {"x": x[core * BPC:(core + 1) * BPC], "bd": bdm, "tri": tri,
         "msk": msk, "idn": idn}
        for core in range(NCORES)
    ]
    out = run_bass_kernel_spmd(nc, in_maps, list(range(NCORES)), trace=_trace)
    y = np.concatenate([out.results[i]["y"] for i in range(NCORES)], 0)
    if _trace:
        kernel.last_exec_time_ns = out.exec_time_ns
        kernel.last_results = out
    return y

